# revision 3
# baseline (speedup 1.0000x reference)
"""Trainium2 Bass kernel for batched multi-head attention.

Problem: x[16,1024,1024] -> Attention(dim=1024, heads=16, dh=64) -> [16,1024,1024]
Sharding: pure data parallel over batch, 2 batch elements per core, 8 cores.
Each core runs an identical single-core graph (no collectives).

Per-core algorithm (per batch element, n=1024 tokens, d=1024):
  1. DMA x rows, PE-transpose to xT [dim, tok] (matmul contracts partitions).
  2. QK^T projection: QKT[feat, tok] = w_qkv[:, :2048].T @ x.T (+bias per
     partition), computed directly in the transposed layout the attention
     matmuls want.  V projection in natural [tok, feat] layout (+bias along
     free dim via a broadcast tile), stored bf16 with a ones column appended
     (denominator trick).
  3. Per head: S^T[k, q] = K^T.T @ Q^T (fp32r matmuls, K=64 contraction);
     exp((1/8)s) on ScalarE straight out of PSUM into bf16 P^T tiles;
     AV: [V|1].T @ P^T accumulated over k-tiles -> [65, q] PSUM where row 64
     is the softmax denominator; reciprocal + K=1 broadcast matmul + DVE
     multiply produce normalized AO^T[inner, tok] -- exactly the stationary
     layout the output projection needs.
  4. Output projection out[tok, d] = AO^T.T @ w_out (+bias), DMA to DRAM.
"""

import numpy as np

_CACHE = {}

B_PER_CORE = 2
N = 1024
DIM = 1024
HEADS = 16
DH = 64
SCALE = DH ** -0.5
N_CORES = 8


def _build_nc():
    import concourse.bass as bass
    from concourse import bacc, mybir, tile
    from concourse.masks import make_identity
    from contextlib import ExitStack

    f32 = mybir.dt.float32
    f32r = mybir.dt.float32r
    bf16 = mybir.dt.bfloat16
    Exp = mybir.ActivationFunctionType.Exp
    OpAdd = mybir.AluOpType.add
    OpMult = mybir.AluOpType.mult

    nc = bacc.Bacc(None, target_bir_lowering=False)

    x_e = nc.declare_dram_parameter("x", [B_PER_CORE, N, DIM], f32, isOutput=False)
    wq_e = nc.declare_dram_parameter("w_qkv", [DIM, 3 * DIM], f32, isOutput=False)
    bq_e = nc.declare_dram_parameter("b_qkv", [3 * DIM], f32, isOutput=False)
    wo_e = nc.declare_dram_parameter("w_out", [DIM, DIM], f32, isOutput=False)
    bo_e = nc.declare_dram_parameter("b_out", [DIM], f32, isOutput=False)
    out_e = nc.declare_dram_parameter("out", [B_PER_CORE, N, DIM], f32, isOutput=True)

    with tile.TileContext(nc) as tc, ExitStack() as top:
        singles = top.enter_context(tc.tile_pool(name="singles", bufs=1))
        ident = singles.tile([128, 128], f32)
        make_identity(nc, ident)
        ones64 = singles.tile([1, 64], f32)
        nc.vector.memset(ones64, 1.0)

        # b_qkv[0:2048] reshaped to [feat_tile(16) partitions, 128] then
        # PE-transposed to per-partition bias layout [128, 16].
        bqk_sb = singles.tile([128, 16], f32)
        tmpb = singles.tile([128, 128], f32)
        nc.vector.memset(tmpb, 0.0)
        nc.sync.dma_start(
            out=tmpb[0:16, :], in_=bq_e[0 : 2 * DIM].rearrange("(j p) -> j p", j=16)
        )
        with tc.tile_pool(name="psinit", bufs=1, space="PSUM") as psi:
            pb = psi.tile([128, 128], f32)
            nc.tensor.transpose(pb, tmpb, ident)
            nc.vector.tensor_copy(out=bqk_sb, in_=pb[:, 0:16])

        # b_v and b_out broadcast along partitions (bias along the free dim).
        bv_bc = singles.tile([128, DIM], f32)
        bo_bc = singles.tile([128, DIM], f32)
        bv_ap = bq_e[2 * DIM : 3 * DIM]
        nc.gpsimd.dma_start(
            out=bv_bc,
            in_=bass.AP(tensor=bv_ap.tensor, offset=bv_ap.offset, ap=[[0, 128], *bv_ap.ap]),
        )
        bo_ap = bo_e[:]
        nc.gpsimd.dma_start(
            out=bo_bc,
            in_=bass.AP(tensor=bo_ap.tensor, offset=bo_ap.offset, ap=[[0, 128], *bo_ap.ap]),
        )

        for b in range(B_PER_CORE):
            with tc.tile_pool(name=f"qkt{b}", bufs=1) as qktp, \
                 tc.tile_pool(name=f"vv{b}", bufs=1) as vvp, \
                 tc.tile_pool(name=f"aot{b}", bufs=1) as aotp:
                # QKT: [feat part, ftile, tok]; ftile 0..7 = Q, 8..15 = K.
                QKT = qktp.tile([128, 16, N], f32)
                # V with ones column: [tok part, ktile, head, 65]
                Vb = vvp.tile([128, 8, HEADS, DH + 1], bf16)
                AOT = aotp.tile([128, 8, N], f32)
                nc.vector.memset(Vb[:, :, :, DH : DH + 1], 1.0)

                with tc.tile_pool(name=f"xt{b}", bufs=1) as xtp:
                    xT = xtp.tile([128, 8, N], f32)

                    # ---- phase 1: load x, transpose to [dim, tok] ----
                    with tc.tile_pool(name=f"xin{b}", bufs=3) as xip, \
                         tc.tile_pool(name=f"pst{b}", bufs=4, space="PSUM") as pstp:
                        for tt in range(8):
                            xin = xip.tile([128, DIM], f32, tag="xin")
                            nc.sync.dma_start(
                                out=xin, in_=x_e[b, tt * 128 : (tt + 1) * 128, :]
                            )
                            for dt in range(8):
                                ps = pstp.tile([128, 128], f32, tag="pst")
                                nc.tensor.transpose(
                                    ps, xin[:, dt * 128 : (dt + 1) * 128], ident
                                )
                                nc.vector.tensor_copy(
                                    out=xT[:, dt, tt * 128 : (tt + 1) * 128], in_=ps
                                )

                    # ---- phase 2a: QK^T projection ----
                    with tc.tile_pool(name=f"wqk{b}", bufs=4) as wqkp, \
                         tc.tile_pool(name=f"psq{b}", bufs=4, space="PSUM") as psqp:
                        for ft in range(16):
                            pss = [psqp.tile([128, 512], f32, tag="psq", name=f"psq{b}_{ft}_{i}") for i in range(2)]
                            for dt in range(8):
                                wt = wqkp.tile([128, 128], f32, tag="wqk")
                                nc.sync.dma_start(
                                    out=wt,
                                    in_=wq_e[
                                        dt * 128 : (dt + 1) * 128,
                                        ft * 128 : (ft + 1) * 128,
                                    ],
                                )
                                for tcx in range(2):
                                    nc.tensor.matmul(
                                        pss[tcx],
                                        lhsT=wt.bitcast(f32r),
                                        rhs=xT[:, dt, tcx * 512 : (tcx + 1) * 512].bitcast(f32r),
                                        start=(dt == 0),
                                        stop=(dt == 7),
                                    )
                            for tcx in range(2):
                                nc.vector.tensor_scalar_add(
                                    out=QKT[:, ft, tcx * 512 : (tcx + 1) * 512],
                                    in0=pss[tcx],
                                    scalar1=bqk_sb[:, ft : ft + 1],
                                )

                    # ---- phase 2b: V projection (natural layout, bf16) ----
                    with tc.tile_pool(name=f"wv{b}", bufs=8) as wvp, \
                         tc.tile_pool(name=f"psv{b}", bufs=3, space="PSUM") as psvp:
                        for tcx in range(2):
                            wvt = [wvp.tile([128, 512], f32, tag="wv", name=f"wv{b}_{tcx}_{i}") for i in range(8)]
                            for dt in range(8):
                                nc.sync.dma_start(
                                    out=wvt[dt],
                                    in_=wq_e[
                                        dt * 128 : (dt + 1) * 128,
                                        2 * DIM + tcx * 512 : 2 * DIM + (tcx + 1) * 512,
                                    ],
                                )
                            for mt in range(8):
                                psv = psvp.tile([128, 512], f32, tag="psv")
                                for dt in range(8):
                                    nc.tensor.matmul(
                                        psv,
                                        lhsT=xT[:, dt, mt * 128 : (mt + 1) * 128].bitcast(f32r),
                                        rhs=wvt[dt].bitcast(f32r),
                                        start=(dt == 0),
                                        stop=(dt == 7),
                                    )
                                nc.vector.tensor_tensor(
                                    out=Vb[:, mt, tcx * 8 : (tcx + 1) * 8, 0:DH],
                                    in0=psv.rearrange("p (h d) -> p h d", h=8),
                                    in1=bv_bc[:, tcx * 512 : (tcx + 1) * 512].rearrange(
                                        "p (h d) -> p h d", h=8
                                    ),
                                    op=OpAdd,
                                )

                # ---- phase 3: attention ----
                with tc.tile_pool(name=f"pt{b}", bufs=12) as ptp, \
                     tc.tile_pool(name=f"rl{b}", bufs=4) as rlp, \
                     tc.tile_pool(name=f"psst{b}", bufs=2, space="PSUM") as psstp, \
                     tc.tile_pool(name=f"psav{b}", bufs=1, space="PSUM") as psavp, \
                     tc.tile_pool(name=f"psbc{b}", bufs=1, space="PSUM") as psbcp:
                    for h in range(HEADS):
                        koff = (h % 2) * 64
                        fq = h // 2
                        fk = 8 + h // 2
                        pts = []
                        for kt in range(8):
                            st = psstp.tile([128, N], f32, tag="st")
                            for half in range(2):
                                nc.tensor.matmul(
                                    st[:, half * 512 : (half + 1) * 512],
                                    lhsT=QKT[
                                        koff : koff + 64, fk, kt * 128 : (kt + 1) * 128
                                    ].bitcast(f32r),
                                    rhs=QKT[
                                        koff : koff + 64, fq, half * 512 : (half + 1) * 512
                                    ].bitcast(f32r),
                                    start=True,
                                    stop=True,
                                )
                            pt = ptp.tile([128, N], bf16, tag="pt")
                            nc.scalar.activation(out=pt, in_=st, func=Exp, scale=SCALE)
                            pts.append(pt)
                        av = psavp.tile([DH + 1, N], f32, tag="av")
                        for kt in range(8):
                            for half in range(2):
                                nc.tensor.matmul(
                                    av[:, half * 512 : (half + 1) * 512],
                                    lhsT=Vb[:, kt, h, :],
                                    rhs=pts[kt][:, half * 512 : (half + 1) * 512],
                                    start=(kt == 0),
                                    stop=(kt == 7),
                                )
                        rl = rlp.tile([1, N], f32, tag="rl")
                        nc.vector.reciprocal(out=rl, in_=av[DH : DH + 1, :])
                        bc = psbcp.tile([64, N], f32, tag="bc")
                        for half in range(2):
                            nc.tensor.matmul(
                                bc[:, half * 512 : (half + 1) * 512],
                                lhsT=ones64.bitcast(f32r),
                                rhs=rl[:, half * 512 : (half + 1) * 512].bitcast(f32r),
                                start=True,
                                stop=True,
                            )
                        nc.vector.tensor_tensor(
                            out=AOT[koff : koff + 64, fq, :],
                            in0=av[0:DH, :],
                            in1=bc,
                            op=OpMult,
                        )

                # ---- phase O: output projection ----
                with tc.tile_pool(name=f"wo{b}", bufs=8) as wop, \
                     tc.tile_pool(name=f"oo{b}", bufs=4) as oop, \
                     tc.tile_pool(name=f"pso{b}", bufs=4, space="PSUM") as psop:
                    for tcx in range(2):
                        wot = [wop.tile([128, 512], f32, tag="wo", name=f"wo{b}_{tcx}_{i}") for i in range(8)]
                        for kt in range(8):
                            nc.sync.dma_start(
                                out=wot[kt],
                                in_=wo_e[
                                    kt * 128 : (kt + 1) * 128,
                                    tcx * 512 : (tcx + 1) * 512,
                                ],
                            )
                        for mt in range(8):
                            pso = psop.tile([128, 512], f32, tag="pso")
                            for kt in range(8):
                                nc.tensor.matmul(
                                    pso,
                                    lhsT=AOT[:, kt, mt * 128 : (mt + 1) * 128].bitcast(f32r),
                                    rhs=wot[kt].bitcast(f32r),
                                    start=(kt == 0),
                                    stop=(kt == 7),
                                )
                            oo = oop.tile([128, 512], f32, tag="oo")
                            nc.vector.tensor_tensor(
                                out=oo,
                                in0=pso,
                                in1=bo_bc[:, tcx * 512 : (tcx + 1) * 512],
                                op=OpAdd,
                            )
                            nc.sync.dma_start(
                                out=out_e[
                                    b,
                                    mt * 128 : (mt + 1) * 128,
                                    tcx * 512 : (tcx + 1) * 512,
                                ],
                                in_=oo,
                            )
    return nc


def get_nc():
    if "nc" not in _CACHE:
        nc = _build_nc()
        nc.finalize()
        _CACHE["nc"] = nc
    return _CACHE["nc"]


def make_in_maps(inputs):
    x = np.ascontiguousarray(np.asarray(inputs["x"], dtype=np.float32))
    w_qkv = np.ascontiguousarray(np.asarray(inputs["w_qkv"], dtype=np.float32))
    b_qkv = np.ascontiguousarray(np.asarray(inputs["b_qkv"], dtype=np.float32))
    w_out = np.ascontiguousarray(np.asarray(inputs["w_out"], dtype=np.float32))
    b_out = np.ascontiguousarray(np.asarray(inputs["b_out"], dtype=np.float32))
    in_maps = []
    for c in range(N_CORES):
        in_maps.append(
            {
                "x": np.ascontiguousarray(x[c * B_PER_CORE : (c + 1) * B_PER_CORE]),
                "w_qkv": w_qkv,
                "b_qkv": b_qkv,
                "w_out": w_out,
                "b_out": b_out,
            }
        )
    return in_maps


def run(inputs, trace=False, **kw):
    from concourse.bass_utils import run_bass_kernel_spmd

    nc = get_nc()
    in_maps = make_in_maps(inputs)
    res = run_bass_kernel_spmd(
        nc, in_maps, core_ids=list(range(N_CORES)), trace=trace, **kw
    )
    out = np.concatenate([res.results[c]["out"] for c in range(N_CORES)], axis=0)
    return out, res


def kernel(**inputs):
    out, _ = run(inputs, trace=False)
    return out
